# revision 4
# baseline (speedup 1.0000x reference)
"""MoE (top-2, GShard-style capacity routing) kernel for 8 Trainium2 NeuronCores.

Expert-parallel: core e owns expert e's MLP (wi/wo/bi/bo slices). Every core
receives the full token stream, computes the (replicated, cheap) router +
top-2 + capacity assignment on device, dispatches its own expert's tokens via
a one-hot matmul, runs the expert MLP in bf16, and scatters back a weighted
partial output y_e. The host sums the 8 partials and takes the aux loss from
core 0.

Router precision: x and w_router are split on the host into bf16 hi/lo planes
(x = xh + xl with |x - xh - xl| <= 2^-18 |x|); logits are computed as
xh@wh + xh@wl + xl@wh in fp32 PSUM, giving ~2^-18-accurate logits so top-k
decisions match the fp32 reference. The bf16 hi planes are reused as the
dispatch/GEMM activations. xT tiles come in via DMA XBAR transpose (2-byte),
which keeps the PE free of fp32 transposes.

Self-contained: hardcodes all shapes; only imports system-installed
concourse/bass.
"""

import numpy as np
import ml_dtypes

# Problem shapes
E = 8
KSEL = 2
D = 1024
M = 4096
GS = 1024
G = 8
CAP = 256
NT = G * GS  # 8192 tokens
N_CORES = 8
P = 128

_CACHE = {}


def _build_nc():
    import concourse.bass as bass
    import concourse.mybir as mybir
    from concourse import bacc
    import concourse.tile as tile
    from concourse.masks import make_identity

    f32 = mybir.dt.float32
    bf16 = mybir.dt.bfloat16
    u32 = mybir.dt.uint32
    Alu = mybir.AluOpType
    Act = mybir.ActivationFunctionType
    Ax = mybir.AxisListType

    nc = bacc.Bacc(None, target_bir_lowering=False, debug=False)

    xhi = nc.declare_dram_parameter("xhi", [NT, D], bf16, isOutput=False)
    xlo = nc.declare_dram_parameter("xlo", [NT, D], bf16, isOutput=False)
    xll = nc.declare_dram_parameter("xll", [NT, D], bf16, isOutput=False)
    wrh = nc.declare_dram_parameter("wrh", [D, E], bf16, isOutput=False)
    wrl = nc.declare_dram_parameter("wrl", [D, E], bf16, isOutput=False)
    wrll = nc.declare_dram_parameter("wrll", [D, E], bf16, isOutput=False)
    wib = nc.declare_dram_parameter("wib", [D, M], bf16, isOutput=False)
    bi_ = nc.declare_dram_parameter("bi", [1, M], f32, isOutput=False)
    wob = nc.declare_dram_parameter("wob", [M, D], bf16, isOutput=False)
    bo_ = nc.declare_dram_parameter("bo", [1, D], f32, isOutput=False)
    eid = nc.declare_dram_parameter("eid", [1, 1], f32, isOutput=False)
    y = nc.declare_dram_parameter("y", [NT, D], f32, isOutput=True)
    aux = nc.declare_dram_parameter("aux", [1, 1], f32, isOutput=True)

    iota_d = nc.inline_tensor(
        np.arange(CAP, dtype=np.float32).reshape(1, CAP), name="iotac"
    )

    with tile.TileContext(nc) as tc:
        cst = tc.alloc_tile_pool(name="cst", bufs=1)
        pers = tc.alloc_tile_pool(name="pers", bufs=1)
        psum = tc.alloc_tile_pool(name="psum", bufs=8, space="PSUM")

        I128f = cst.tile([P, P], f32, tag="I128f")
        make_identity(nc, I128f)
        I128b = cst.tile([P, P], bf16, tag="I128b")
        make_identity(nc, I128b)
        iota256 = cst.tile([P, CAP], f32, tag="iota256")
        nc.sync.dma_start(iota256[:], iota_d[:, :].to_broadcast((P, CAP)))
        bo_b = cst.tile([P, D], f32, tag="bo_b")
        nc.sync.dma_start(bo_b[:], bo_[:, :].to_broadcast((P, D)))
        ones_col = cst.tile([P, 1], f32, tag="ones_col")
        nc.vector.memset(ones_col[:], 1.0)
        bi_sb = cst.tile([P, M // P], f32, tag="bi_sb")
        nc.sync.dma_start(bi_sb[:], bi_[:, :].rearrange("a (mi p) -> p (a mi)", p=P))
        wrh_sb = cst.tile([P, D // P, E], bf16, tag="wrh_sb")
        nc.sync.dma_start(wrh_sb[:], wrh[:, :].rearrange("(dc p) e -> p dc e", p=P))
        wrl_sb = cst.tile([P, D // P, E], bf16, tag="wrl_sb")
        nc.sync.dma_start(wrl_sb[:], wrl[:, :].rearrange("(dc p) e -> p dc e", p=P))
        wrll_sb = cst.tile([P, D // P, E], bf16, tag="wrll_sb")
        nc.sync.dma_start(wrll_sb[:], wrll[:, :].rearrange("(dc p) e -> p dc e", p=P))
        ecol8 = cst.tile([G, 1], f32, tag="ecol8")
        nc.sync.dma_start(ecol8[:], eid[:, :].to_broadcast((G, 1)))

        poscols = pers.tile([P, 8, G], f32, tag="poscols")
        wcols = pers.tile([P, 8, G], f32, tag="wcols")
        impT = pers.tile([E, G], f32, tag="impT")

        # ------------------------------------------------------------------
        # Phase 1: router (split-bf16, fp32-accurate), top-2, capacity scan
        # ------------------------------------------------------------------
        rt = tc.alloc_tile_pool(name="rt", bufs=1)
        ph1 = tc.alloc_tile_pool(name="ph1", bufs=3)
        xtp = tc.alloc_tile_pool(name="xtp", bufs=2)

        rstage = rt.tile([4, NT], f32, tag="rstage")
        imp_ps = {}

        for st in range(16):  # 512-token tiles
            xTh = xtp.tile([P, D // P, 512], bf16, tag="xTh")
            xTl = xtp.tile([P, D // P, 512], bf16, tag="xTl")
            xTll = xtp.tile([P, D // P, 512], bf16, tag="xTll")
            for dc in range(D // P):
                nc.scalar.dma_start_transpose(
                    xTh[:, dc, :], xhi[st * 512 : (st + 1) * 512, dc * P : (dc + 1) * P]
                )
                nc.scalar.dma_start_transpose(
                    xTl[:, dc, :], xlo[st * 512 : (st + 1) * 512, dc * P : (dc + 1) * P]
                )
                nc.scalar.dma_start_transpose(
                    xTll[:, dc, :], xll[st * 512 : (st + 1) * 512, dc * P : (dc + 1) * P]
                )
            pslt = psum.tile([E, 512], f32, tag="bank")
            terms = (
                (wrh_sb, xTh), (wrl_sb, xTh), (wrll_sb, xTh),
                (wrh_sb, xTl), (wrl_sb, xTl), (wrh_sb, xTll),
            )
            k = 0
            for dc in range(D // P):
                for wt, xt in terms:
                    nc.tensor.matmul(
                        pslt[:], wt[:, dc, :], xt[:, dc, :],
                        start=(k == 0), stop=(k == len(terms) * D // P - 1),
                    )
                    k += 1
            ltT = ph1.tile([E, 512], f32, tag="ltT")
            nc.vector.tensor_copy(ltT[:], pslt[:])
            for q in range(4):
                sc = st * 4 + q
                g, j = sc // 8, sc % 8
                psl = psum.tile([P, E], f32, tag="bank")
                nc.tensor.transpose(psl[:], ltT[:, q * P : (q + 1) * P], I128f[:E, :E])
                lsb = ph1.tile([P, E], f32, tag="lsb")
                nc.vector.tensor_copy(lsb[:], psl[:])
                nm = ph1.tile([P, 1], f32, tag="nm")
                nc.vector.tensor_reduce(nm[:], lsb[:], axis=Ax.X, op=Alu.max, negate=True)
                m8 = ph1.tile([P, E], f32, tag="m8")
                nc.vector.max(m8[:], lsb[:])
                i8 = ph1.tile([P, E], u32, tag="i8")
                nc.vector.max_index(i8[:], m8[:], lsb[:])
                gat = ph1.tile([P, E], f32, tag="gat")
                se = ph1.tile([P, 1], f32, tag="se")
                nc.scalar.activation(gat[:], psl[:], Act.Exp, bias=nm[:], accum_out=se[:])
                rc = ph1.tile([P, 1], f32, tag="rc")
                nc.vector.reciprocal(rc[:], se[:])
                nc.vector.tensor_scalar_mul(gat[:], gat[:], rc[:])
                if j == 0:
                    imp_ps[g] = psum.tile([E, 1], f32, tag="bank", name=f"imp{g}")
                nc.tensor.matmul(
                    imp_ps[g][:], gat[:], ones_col[:], start=(j == 0), stop=(j == 7)
                )
                tw2 = ph1.tile([P, KSEL], f32, tag="tw2")
                nc.scalar.activation(tw2[:], m8[:, 0:KSEL], Act.Exp, bias=nm[:])
                pk = ph1.tile([P, 4], f32, tag="pk")
                nc.vector.tensor_copy(pk[:, 0:2], i8[:, 0:2])
                nc.vector.tensor_scalar_mul(pk[:, 2:4], tw2[:], rc[:])
                pspk = psum.tile([4, P], f32, tag="bank")
                nc.tensor.transpose(pspk[:], pk[:], I128f[:])
                nc.vector.tensor_copy(rstage[0:4, sc * P : (sc + 1) * P], pspk[:])
                if j == 7:
                    nc.vector.tensor_copy(impT[:, g : g + 1], imp_ps[g][:])

        # k-major stream [g, t=k*GS+s] and capacity scan
        topiT = rt.tile([G, KSEL * GS], f32, tag="topiT")
        twT = rt.tile([G, KSEL * GS], f32, tag="twT")
        nc.sync.dma_start(topiT[:, 0:GS], rstage[0:1, :])
        nc.sync.dma_start(topiT[:, GS : 2 * GS], rstage[1:2, :])
        nc.sync.dma_start(twT[:, 0:GS], rstage[2:3, :])
        nc.sync.dma_start(twT[:, GS : 2 * GS], rstage[3:4, :])
        zz8 = rt.tile([G, KSEL * GS], f32, tag="zz8")
        nc.vector.memset(zz8[:], 0.0)
        ohh = rt.tile([G, KSEL * GS], f32, tag="ohh")
        nc.vector.tensor_scalar(ohh[:], topiT[:], ecol8[:, 0:1], None, op0=Alu.is_equal)
        incl = rt.tile([G, KSEL * GS], f32, tag="incl")
        nc.vector.tensor_tensor_scan(incl[:], ohh[:], zz8[:], 0.0, op0=Alu.add, op1=Alu.add)
        pos = rt.tile([G, KSEL * GS], f32, tag="pos")
        nc.vector.tensor_tensor(pos[:], incl[:], ohh[:], Alu.subtract)
        keep = rt.tile([G, KSEL * GS], f32, tag="keep")
        nc.vector.scalar_tensor_tensor(
            keep[:], in0=pos[:], scalar=float(CAP), in1=ohh[:],
            op0=Alu.is_lt, op1=Alu.mult,
        )
        posm = rt.tile([G, KSEL * GS], f32, tag="posm")
        nc.vector.scalar_tensor_tensor(
            posm[:], in0=pos[:], scalar=999.0, in1=keep[:],
            op0=Alu.subtract, op1=Alu.mult,
        )
        nc.vector.tensor_scalar_add(posm[:], posm[:], 999.0)
        wsel = rt.tile([G, KSEL * GS], f32, tag="wsel")
        nc.vector.tensor_tensor(wsel[:], twT[:], keep[:], Alu.mult)
        posm_tok = rt.tile([G, GS], f32, tag="posm_tok")
        nc.vector.tensor_tensor(posm_tok[:], posm[:, 0:GS], posm[:, GS : 2 * GS], Alu.min)
        wtok = rt.tile([G, GS], f32, tag="wtok")
        nc.vector.tensor_tensor(wtok[:], wsel[:, 0:GS], wsel[:, GS : 2 * GS], Alu.add)
        for j in range(8):
            p1 = psum.tile([P, G], f32, tag="bank")
            nc.tensor.transpose(p1[:], posm_tok[:, j * P : (j + 1) * P], I128f[:G, :G])
            nc.vector.tensor_copy(poscols[:, j, :], p1[:])
            p2 = psum.tile([P, G], f32, tag="bank")
            nc.tensor.transpose(p2[:], wtok[:, j * P : (j + 1) * P], I128f[:G, :G])
            nc.vector.tensor_copy(wcols[:, j, :], p2[:])

        # aux = mean_g (std_e(imp)/mean_e(imp))^2
        pst = psum.tile([G, E], f32, tag="bank")
        nc.tensor.transpose(pst[:], impT[:], I128f[:E, :E])
        imp_ge = pers.tile([G, E], f32, tag="imp_ge")
        nc.vector.tensor_copy(imp_ge[:], pst[:])
        mu = pers.tile([G, 1], f32, tag="mu")
        nc.vector.tensor_reduce(mu[:], imp_ge[:], axis=Ax.X, op=Alu.add)
        nc.vector.tensor_scalar_mul(mu[:], mu[:], 1.0 / E)
        dif = pers.tile([G, E], f32, tag="dif")
        nc.vector.tensor_scalar(dif[:], imp_ge[:], mu[:, 0:1], None, op0=Alu.subtract)
        nc.vector.tensor_tensor(dif[:], dif[:], dif[:], Alu.mult)
        var = pers.tile([G, 1], f32, tag="var")
        nc.vector.tensor_reduce(var[:], dif[:], axis=Ax.X, op=Alu.add)
        nc.vector.tensor_scalar_mul(var[:], var[:], 1.0 / E)
        mu2 = pers.tile([G, 1], f32, tag="mu2")
        nc.vector.tensor_tensor(mu2[:], mu[:], mu[:], Alu.mult)
        rr = pers.tile([G, 1], f32, tag="rr")
        nc.vector.reciprocal(rr[:], mu2[:])
        ratio = pers.tile([G, 1], f32, tag="ratio")
        nc.vector.tensor_tensor(ratio[:], var[:], rr[:], Alu.mult)
        psa = psum.tile([1, G], f32, tag="bank")
        nc.tensor.transpose(psa[:], ratio[:], I128f[:G, :G])
        arow = pers.tile([1, G], f32, tag="arow")
        nc.vector.tensor_copy(arow[:], psa[:])
        auxv = pers.tile([1, 1], f32, tag="auxv")
        nc.vector.tensor_reduce(auxv[:], arow[:], axis=Ax.X, op=Alu.add)
        nc.vector.tensor_scalar_mul(auxv[:], auxv[:], 1.0 / G)
        nc.sync.dma_start(aux[:, :], auxv[:])

        xtp.release()
        ph1.release()
        rt.release()

        # ------------------------------------------------------------------
        # Phase 2: dispatch -> MLP -> combine, two halves of 4 groups
        # ------------------------------------------------------------------
        xep = tc.alloc_tile_pool(name="xep", bufs=1)
        hp = tc.alloc_tile_pool(name="hp", bufs=1)
        xbp = tc.alloc_tile_pool(name="xbp", bufs=3)
        dpp = tc.alloc_tile_pool(name="dpp", bufs=3)
        wip = tc.alloc_tile_pool(name="wip", bufs=3)
        wop = tc.alloc_tile_pool(name="wop", bufs=3)
        yep = tc.alloc_tile_pool(name="yep", bufs=1)
        ctp = tc.alloc_tile_pool(name="ctp", bufs=2)
        cbp = tc.alloc_tile_pool(name="cbp", bufs=3)
        yop = tc.alloc_tile_pool(name="yop", bufs=3)

        for hh in range(2):
            # dispatch: xeT[d, dc, gi, c] = x^T gathered per capacity slot
            xeT = xep.tile([P, D // P, 4, CAP], bf16, tag="xeT")
            for gi in range(4):
                g = 4 * hh + gi
                psxe = [
                    psum.tile([P, CAP], f32, tag="bank", name=f"xe{g}_{dc}")
                    for dc in range(D // P)
                ]
                for jj in range(4):
                    xb = xbp.tile([P, 2, D], bf16, tag="xb")
                    base = (g * 8 + jj * 2) * P
                    nc.sync.dma_start(
                        xb[:],
                        xhi[base : base + 2 * P, :].rearrange("(two p) d -> p two d", p=P),
                    )
                    for j2 in range(2):
                        j = jj * 2 + j2
                        dp = dpp.tile([P, CAP], bf16, tag="dp")
                        nc.vector.tensor_scalar(
                            dp[:], iota256[:], poscols[:, j, g : g + 1], None,
                            op0=Alu.is_equal,
                        )
                        for dc in range(D // P):
                            nc.tensor.matmul(
                                psxe[dc][:], xb[:, j2, dc * P : (dc + 1) * P], dp[:],
                                start=(j == 0), stop=(j == 7),
                            )
                for dc in range(D // P):
                    nc.any.tensor_copy(xeT[:, dc, gi, :], psxe[dc][:])

            # GEMM1 + bias + gelu -> h[m, mi, t]  (t = gi*256 + c, 1024 per half)
            h_t = hp.tile([P, M // P, 4 * CAP], bf16, tag="h_t")
            for mi in range(M // P):
                wib_t = wip.tile([P, D // P, P], bf16, tag="wib_t")
                nc.sync.dma_start(
                    wib_t[:],
                    wib[:, mi * P : (mi + 1) * P].rearrange("(dc p) m -> p dc m", p=P),
                )
                for pr in range(2):
                    psh = psum.tile([P, 2 * CAP], f32, tag="bank")
                    for dc in range(D // P):
                        nc.tensor.matmul(
                            psh[:], wib_t[:, dc, :], xeT[:, dc, pr * 2 : pr * 2 + 2, :],
                            start=(dc == 0), stop=(dc == D // P - 1),
                        )
                    nc.scalar.activation(
                        h_t[:, mi, pr * 512 : (pr + 1) * 512], psh[:],
                        Act.Gelu_apprx_tanh, bias=bi_sb[:, mi : mi + 1],
                    )

            # GEMM2 and combine, 2 sets x 512 tokens
            for st in range(2):
                psye = [
                    [
                        psum.tile([P, 512], f32, tag="bank", name=f"ye{hh}_{st}_{tc_}_{dt}")
                        for dt in range(2)
                    ]
                    for tc_ in range(4)
                ]
                for mi in range(M // P):
                    wob_t = wop.tile([P, D], bf16, tag="wob_t")
                    nc.sync.dma_start(wob_t[:], wob[mi * P : (mi + 1) * P, :])
                    for tc_ in range(4):
                        for dt in range(2):
                            nc.tensor.matmul(
                                psye[tc_][dt][:],
                                h_t[:, mi, st * 512 + tc_ * P : st * 512 + (tc_ + 1) * P],
                                wob_t[:, dt * 512 : (dt + 1) * 512],
                                start=(mi == 0), stop=(mi == M // P - 1),
                            )
                ye_sb = yep.tile([P, 4, D], bf16, tag="ye_sb")
                for tc_ in range(4):
                    for dt in range(2):
                        nc.any.tensor_copy(
                            ye_sb[:, tc_, dt * 512 : (dt + 1) * 512], psye[tc_][dt][:]
                        )
                for g2 in range(2):
                    g = 4 * hh + 2 * st + g2
                    cT = ctp.tile([P, 2, GS], bf16, tag="cT")
                    for j in range(8):
                        cb = cbp.tile([P, CAP], bf16, tag="cb")
                        nc.vector.tensor_scalar(
                            cb[:], iota256[:], poscols[:, j, g : g + 1], None,
                            op0=Alu.is_equal,
                        )
                        nc.vector.tensor_scalar_mul(cb[:], cb[:], wcols[:, j, g : g + 1])
                        for cc in range(2):
                            pcb = psum.tile([P, P], bf16, tag="bank")
                            nc.tensor.transpose(
                                pcb[:], cb[:, cc * P : (cc + 1) * P], I128b[:]
                            )
                            nc.any.tensor_copy(cT[:, cc, j * P : (j + 1) * P], pcb[:])
                    for j in range(8):
                        yt = yop.tile([P, D], f32, tag="yt")
                        for dt in range(2):
                            psy = psum.tile([P, 512], f32, tag="bank")
                            for cc in range(2):
                                nc.tensor.matmul(
                                    psy[:],
                                    cT[:, cc, j * P : (j + 1) * P],
                                    ye_sb[:, g2 * 2 + cc, dt * 512 : (dt + 1) * 512],
                                    start=(cc == 0), stop=(cc == 1),
                                )
                            nc.vector.scalar_tensor_tensor(
                                yt[:, dt * 512 : (dt + 1) * 512],
                                in0=bo_b[:, dt * 512 : (dt + 1) * 512],
                                scalar=wcols[:, j, g : g + 1], in1=psy[:],
                                op0=Alu.mult, op1=Alu.add,
                            )
                        nc.sync.dma_start(
                            y[g * GS + j * P : g * GS + (j + 1) * P, :], yt[:]
                        )

        for pool in (yop, cbp, ctp, yep, wop, wip, dpp, xbp, hp, xep, psum, pers, cst):
            pool.release()

    nc.compile()
    return nc


def _get_nc():
    if "nc" not in _CACHE:
        _CACHE["nc"] = _build_nc()
    return _CACHE["nc"]


def kernel(**inputs):
    from concourse.bass_utils import run_bass_kernel_spmd

    nc = _get_nc()

    x = np.ascontiguousarray(np.asarray(inputs["inputs"], dtype=np.float32)).reshape(NT, D)
    w_router = np.ascontiguousarray(np.asarray(inputs["w_router"], dtype=np.float32))
    wi = np.asarray(inputs["wi"], dtype=np.float32)
    bi = np.asarray(inputs["bi"], dtype=np.float32)
    wo = np.asarray(inputs["wo"], dtype=np.float32)
    bo = np.asarray(inputs["bo"], dtype=np.float32)

    x_hi = x.astype(ml_dtypes.bfloat16)
    x_lo = (x - x_hi.astype(np.float32)).astype(ml_dtypes.bfloat16)
    x_ll = (x - x_hi.astype(np.float32) - x_lo.astype(np.float32)).astype(ml_dtypes.bfloat16)
    wr_hi = w_router.astype(ml_dtypes.bfloat16)
    wr_lo = (w_router - wr_hi.astype(np.float32)).astype(ml_dtypes.bfloat16)
    wr_ll = (
        w_router - wr_hi.astype(np.float32) - wr_lo.astype(np.float32)
    ).astype(ml_dtypes.bfloat16)
    wi_bf = wi.astype(ml_dtypes.bfloat16)
    wo_bf = wo.astype(ml_dtypes.bfloat16)

    in_maps = []
    for e in range(N_CORES):
        in_maps.append(
            {
                "xhi": x_hi,
                "xlo": x_lo,
                "xll": x_ll,
                "wrh": wr_hi,
                "wrl": wr_lo,
                "wrll": wr_ll,
                "wib": np.ascontiguousarray(wi_bf[e]),
                "bi": np.ascontiguousarray(bi[e]).reshape(1, M),
                "wob": np.ascontiguousarray(wo_bf[e]),
                "bo": np.ascontiguousarray(bo[e]).reshape(1, D),
                "eid": np.full((1, 1), float(e), dtype=np.float32),
            }
        )

    trace = bool(_CACHE.get("trace", False))
    res = run_bass_kernel_spmd(
        nc, in_maps, core_ids=list(range(N_CORES)), trace=trace,
        tmpdir=_CACHE.get("tmpdir"),
    )
    _CACHE["exec_time_ns"] = res.exec_time_ns

    y = res.results[0]["y"].astype(np.float64)
    for e in range(1, N_CORES):
        y += res.results[e]["y"]
    y = y.astype(np.float32).reshape(np.asarray(inputs["inputs"]).shape)
    aux = np.float32(res.results[0]["aux"][0, 0])
    return y, aux


# revision 11
# speedup vs baseline: 1.4418x; 1.4418x over previous
"""MoE (top-2, GShard-style capacity routing) kernel for 8 Trainium2 NeuronCores.

Expert-parallel: core e owns expert e's MLP (wi/wo/bi/bo slices). Every core
receives the full token stream, computes the (replicated, cheap) router +
top-2 + capacity assignment on device, dispatches its own expert's tokens via
a one-hot matmul, runs the expert MLP in bf16, and scatters back a weighted
partial output y_e. The host sums the 8 partials and takes the aux loss from
core 0.

Router precision: x and w_router are split on the host into bf16 hi/lo planes
(x = xh + xl with |x - xh - xl| <= 2^-18 |x|); logits are computed as
xh@wh + xh@wl + xl@wh in fp32 PSUM, giving ~2^-18-accurate logits so top-k
decisions match the fp32 reference. The bf16 hi planes are reused as the
dispatch/GEMM activations. xT tiles come in via DMA XBAR transpose (2-byte),
which keeps the PE free of fp32 transposes.

Self-contained: hardcodes all shapes; only imports system-installed
concourse/bass.
"""

import numpy as np
import ml_dtypes

# Problem shapes
E = 8
KSEL = 2
D = 1024
M = 4096
GS = 1024
G = 8
CAP = 256
NT = G * GS  # 8192 tokens
N_CORES = 8
P = 128

_CACHE = {}


def _build_nc():
    import concourse.bass as bass
    import concourse.mybir as mybir
    from concourse import bacc
    import concourse.tile as tile
    from concourse.masks import make_identity

    f32 = mybir.dt.float32
    bf16 = mybir.dt.bfloat16
    u32 = mybir.dt.uint32
    Alu = mybir.AluOpType
    Act = mybir.ActivationFunctionType
    Ax = mybir.AxisListType

    nc = bacc.Bacc(None, target_bir_lowering=False, debug=False)

    xhi = nc.declare_dram_parameter("xhi", [NT, D], bf16, isOutput=False)
    xt32 = nc.declare_dram_parameter("xt32", [D, NT], f32, isOutput=False)
    wr = nc.declare_dram_parameter("wr", [D, E], f32, isOutput=False)
    wib = nc.declare_dram_parameter("wib", [D, M], bf16, isOutput=False)
    bi_ = nc.declare_dram_parameter("bi", [1, M], f32, isOutput=False)
    wob = nc.declare_dram_parameter("wob", [M, D], bf16, isOutput=False)
    bo_ = nc.declare_dram_parameter("bo", [1, D], f32, isOutput=False)
    eid = nc.declare_dram_parameter("eid", [1, 1], f32, isOutput=False)
    y = nc.declare_dram_parameter("y", [NT, D], f32, isOutput=True)
    aux = nc.declare_dram_parameter("aux", [1, 1], f32, isOutput=True)

    iota_d = nc.inline_tensor(
        np.arange(CAP, dtype=np.float32).reshape(1, CAP), name="iotac"
    )

    with tile.TileContext(nc) as tc:
        cst = tc.alloc_tile_pool(name="cst", bufs=1)
        pers = tc.alloc_tile_pool(name="pers", bufs=1)
        psum = tc.alloc_tile_pool(name="psum", bufs=8, space="PSUM")

        I128f = cst.tile([P, P], f32, tag="I128f")
        make_identity(nc, I128f)
        I128b = cst.tile([P, P], bf16, tag="I128b")
        make_identity(nc, I128b)
        iota256 = cst.tile([P, CAP], f32, tag="iota256")
        nc.sync.dma_start(iota256[:], iota_d[:, :].to_broadcast((P, CAP)))
        bo_b = cst.tile([P, D], f32, tag="bo_b")
        nc.sync.dma_start(bo_b[:], bo_[:, :].to_broadcast((P, D)))
        ones_col = cst.tile([P, 1], f32, tag="ones_col")
        nc.vector.memset(ones_col[:], 1.0)
        bi_sb = cst.tile([P, M // P], f32, tag="bi_sb")
        nc.sync.dma_start(bi_sb[:], bi_[:, :].rearrange("a (mi p) -> p (a mi)", p=P))
        wr_sb = cst.tile([P, D // P, E], f32, tag="wr_sb")
        nc.sync.dma_start(wr_sb[:], wr[:, :].rearrange("(dc p) e -> p dc e", p=P))
        ecol8 = cst.tile([G, 1], f32, tag="ecol8")
        nc.sync.dma_start(ecol8[:], eid[:, :].to_broadcast((G, 1)))

        poscols = pers.tile([P, 8, G], f32, tag="poscols")
        wcols = pers.tile([P, 8, G], f32, tag="wcols")
        impT = pers.tile([E, G], f32, tag="impT")

        # ------------------------------------------------------------------
        # Phase 1: router (split-bf16, fp32-accurate), top-2, capacity scan
        # ------------------------------------------------------------------
        rt = tc.alloc_tile_pool(name="rt", bufs=1)
        ph1 = tc.alloc_tile_pool(name="ph1", bufs=3)
        xtp = tc.alloc_tile_pool(name="xtp", bufs=2)

        rstage = rt.tile([4, NT], f32, tag="rstage")
        gatp = tc.alloc_tile_pool(name="gatp", bufs=12)
        imp_ps = {}
        ltTs = {}
        deferred = {}

        def softmax_block(st):
            ltT = ltTs.pop(st)
            defer = []
            for q in range(4):
                sc = st * 4 + q
                psl = psum.tile([P, E], f32, tag="bank")
                nc.tensor.transpose(psl[:], ltT[:, q * P : (q + 1) * P], I128f[:E, :E])
                lsb = ph1.tile([P, E], f32, tag="lsb")
                nc.vector.tensor_copy(lsb[:], psl[:])
                nm = ph1.tile([P, 1], f32, tag="nm")
                nc.vector.tensor_reduce(nm[:], lsb[:], axis=Ax.X, op=Alu.max, negate=True)
                m8 = ph1.tile([P, E], f32, tag="m8")
                nc.vector.max(m8[:], lsb[:])
                i8 = ph1.tile([P, E], u32, tag="i8")
                nc.vector.max_index(i8[:], m8[:], lsb[:])
                gat = gatp.tile([P, E], f32, tag="gat")
                se = ph1.tile([P, 1], f32, tag="se")
                nc.scalar.activation(gat[:], psl[:], Act.Exp, bias=nm[:], accum_out=se[:])
                rc = gatp.tile([P, 1], f32, tag="rc")
                nc.vector.reciprocal(rc[:], se[:])
                tw2 = ph1.tile([P, KSEL], f32, tag="tw2")
                nc.scalar.activation(tw2[:], m8[:, 0:KSEL], Act.Exp, bias=nm[:])
                pk = gatp.tile([P, 4], f32, tag="pk")
                nc.vector.tensor_copy(pk[:, 0:2], i8[:, 0:2])
                nc.vector.tensor_scalar_mul(pk[:, 2:4], tw2[:], rc[:])
                defer.append((sc, gat, rc, pk))
            deferred[st] = defer

        def pe_block(st):
            for sc, gat, rc, pk in deferred.pop(st):
                g, j = sc // 8, sc % 8
                if j == 0:
                    imp_ps[g] = psum.tile([E, 1], f32, tag="bank", name=f"imp{g}")
                # imp accumulates sum_s exp/sumexp via rhs = 1/sumexp column
                nc.tensor.matmul(
                    imp_ps[g][:], gat[:], rc[:], start=(j == 0), stop=(j == 7)
                )
                pspk = psum.tile([4, P], f32, tag="bank")
                nc.tensor.transpose(pspk[:], pk[:], I128f[:])
                nc.vector.tensor_copy(rstage[0:4, sc * P : (sc + 1) * P], pspk[:])
                if j == 7:
                    nc.vector.tensor_copy(impT[:, g : g + 1], imp_ps[g][:])

        for st in range(16):  # 512-token tiles, software-pipelined
            xT = xtp.tile([P, D // P, 512], f32, tag="xT")
            nc.scalar.dma_start(
                xT[:],
                xt32[:, st * 512 : (st + 1) * 512].rearrange("(dc p) s -> p dc s", p=P),
            )
            pslt = psum.tile([E, 512], f32, tag="bank")
            for dc in range(D // P):
                nc.tensor.matmul(
                    pslt[:], wr_sb[:, dc, :], xT[:, dc, :],
                    start=(dc == 0), stop=(dc == D // P - 1),
                )
            ltT = ph1.tile([E, 512], f32, tag="ltT")
            nc.vector.tensor_copy(ltT[:], pslt[:])
            ltTs[st] = ltT
            if st >= 1:
                softmax_block(st - 1)
            if st >= 2:
                pe_block(st - 2)
        softmax_block(15)
        pe_block(14)
        pe_block(15)
        gatp.release()

        # k-major stream [g, t=k*GS+s] and capacity scan
        topiT = rt.tile([G, KSEL * GS], f32, tag="topiT")
        twT = rt.tile([G, KSEL * GS], f32, tag="twT")
        nc.sync.dma_start(topiT[:, 0:GS], rstage[0:1, :])
        nc.sync.dma_start(topiT[:, GS : 2 * GS], rstage[1:2, :])
        nc.sync.dma_start(twT[:, 0:GS], rstage[2:3, :])
        nc.sync.dma_start(twT[:, GS : 2 * GS], rstage[3:4, :])
        zz8 = rt.tile([G, KSEL * GS], f32, tag="zz8")
        nc.vector.memset(zz8[:], 0.0)
        ohh = rt.tile([G, KSEL * GS], f32, tag="ohh")
        nc.vector.tensor_scalar(ohh[:], topiT[:], ecol8[:, 0:1], None, op0=Alu.is_equal)
        incl = rt.tile([G, KSEL * GS], f32, tag="incl")
        nc.vector.tensor_tensor_scan(incl[:], ohh[:], zz8[:], 0.0, op0=Alu.add, op1=Alu.add)
        pos = rt.tile([G, KSEL * GS], f32, tag="pos")
        nc.vector.tensor_tensor(pos[:], incl[:], ohh[:], Alu.subtract)
        keep = rt.tile([G, KSEL * GS], f32, tag="keep")
        nc.vector.scalar_tensor_tensor(
            keep[:], in0=pos[:], scalar=float(CAP), in1=ohh[:],
            op0=Alu.is_lt, op1=Alu.mult,
        )
        posm = rt.tile([G, KSEL * GS], f32, tag="posm")
        nc.vector.scalar_tensor_tensor(
            posm[:], in0=pos[:], scalar=999.0, in1=keep[:],
            op0=Alu.subtract, op1=Alu.mult,
        )
        nc.vector.tensor_scalar_add(posm[:], posm[:], 999.0)
        wsel = rt.tile([G, KSEL * GS], f32, tag="wsel")
        nc.vector.tensor_tensor(wsel[:], twT[:], keep[:], Alu.mult)
        posm_tok = rt.tile([G, GS], f32, tag="posm_tok")
        nc.vector.tensor_tensor(posm_tok[:], posm[:, 0:GS], posm[:, GS : 2 * GS], Alu.min)
        wtok = rt.tile([G, GS], f32, tag="wtok")
        nc.vector.tensor_tensor(wtok[:], wsel[:, 0:GS], wsel[:, GS : 2 * GS], Alu.add)
        for j in range(8):
            p1 = psum.tile([P, G], f32, tag="bank")
            nc.tensor.transpose(p1[:], posm_tok[:, j * P : (j + 1) * P], I128f[:G, :G])
            nc.vector.tensor_copy(poscols[:, j, :], p1[:])
            p2 = psum.tile([P, G], f32, tag="bank")
            nc.tensor.transpose(p2[:], wtok[:, j * P : (j + 1) * P], I128f[:G, :G])
            nc.vector.tensor_copy(wcols[:, j, :], p2[:])

        # aux = mean_g (std_e(imp)/mean_e(imp))^2
        pst = psum.tile([G, E], f32, tag="bank")
        nc.tensor.transpose(pst[:], impT[:], I128f[:E, :E])
        imp_ge = pers.tile([G, E], f32, tag="imp_ge")
        nc.vector.tensor_copy(imp_ge[:], pst[:])
        mu = pers.tile([G, 1], f32, tag="mu")
        nc.vector.tensor_reduce(mu[:], imp_ge[:], axis=Ax.X, op=Alu.add)
        nc.vector.tensor_scalar_mul(mu[:], mu[:], 1.0 / E)
        dif = pers.tile([G, E], f32, tag="dif")
        nc.vector.tensor_scalar(dif[:], imp_ge[:], mu[:, 0:1], None, op0=Alu.subtract)
        nc.vector.tensor_tensor(dif[:], dif[:], dif[:], Alu.mult)
        var = pers.tile([G, 1], f32, tag="var")
        nc.vector.tensor_reduce(var[:], dif[:], axis=Ax.X, op=Alu.add)
        nc.vector.tensor_scalar_mul(var[:], var[:], 1.0 / E)
        mu2 = pers.tile([G, 1], f32, tag="mu2")
        nc.vector.tensor_tensor(mu2[:], mu[:], mu[:], Alu.mult)
        rr = pers.tile([G, 1], f32, tag="rr")
        nc.vector.reciprocal(rr[:], mu2[:])
        ratio = pers.tile([G, 1], f32, tag="ratio")
        nc.vector.tensor_tensor(ratio[:], var[:], rr[:], Alu.mult)
        psa = psum.tile([1, G], f32, tag="bank")
        nc.tensor.transpose(psa[:], ratio[:], I128f[:G, :G])
        arow = pers.tile([1, G], f32, tag="arow")
        nc.vector.tensor_copy(arow[:], psa[:])
        auxv = pers.tile([1, 1], f32, tag="auxv")
        nc.vector.tensor_reduce(auxv[:], arow[:], axis=Ax.X, op=Alu.add)
        nc.vector.tensor_scalar_mul(auxv[:], auxv[:], 1.0 / G)
        nc.sync.dma_start(aux[:, :], auxv[:])

        xtp.release()
        ph1.release()
        rt.release()

        # ------------------------------------------------------------------
        # Phase 2: dispatch -> MLP -> combine, two halves of 4 groups
        # ------------------------------------------------------------------
        xep = tc.alloc_tile_pool(name="xep", bufs=1)
        hp = tc.alloc_tile_pool(name="hp", bufs=1)
        xbp = tc.alloc_tile_pool(name="xbp", bufs=3)
        dpp = tc.alloc_tile_pool(name="dpp", bufs=3)
        wip = tc.alloc_tile_pool(name="wip", bufs=3)
        wop = tc.alloc_tile_pool(name="wop", bufs=3)
        yep = tc.alloc_tile_pool(name="yep", bufs=1)
        ctp = tc.alloc_tile_pool(name="ctp", bufs=2)
        cbp = tc.alloc_tile_pool(name="cbp", bufs=3)
        yop = tc.alloc_tile_pool(name="yop", bufs=3)

        for hh in range(2):
            # dispatch: xeT[d, dc, gi, c] = x^T gathered per capacity slot
            xeT = xep.tile([P, D // P, 4, CAP], bf16, tag="xeT")
            for gi in range(4):
                g = 4 * hh + gi
                psxe = [
                    psum.tile([P, CAP], f32, tag="bank", name=f"xe{g}_{dc}")
                    for dc in range(D // P)
                ]
                for jj in range(4):
                    xb = xbp.tile([P, 2, D], bf16, tag="xb")
                    base = (g * 8 + jj * 2) * P
                    nc.sync.dma_start(
                        xb[:],
                        xhi[base : base + 2 * P, :].rearrange("(two p) d -> p two d", p=P),
                    )
                    for j2 in range(2):
                        j = jj * 2 + j2
                        dp = dpp.tile([P, CAP], bf16, tag="dp")
                        nc.vector.tensor_scalar(
                            dp[:], iota256[:], poscols[:, j, g : g + 1], None,
                            op0=Alu.is_equal,
                        )
                        for dc in range(D // P):
                            nc.tensor.matmul(
                                psxe[dc][:], xb[:, j2, dc * P : (dc + 1) * P], dp[:],
                                start=(j == 0), stop=(j == 7),
                            )
                for dc in range(D // P):
                    nc.any.tensor_copy(xeT[:, dc, gi, :], psxe[dc][:])

            # GEMM1 + bias + gelu -> h[m, mi, t]  (t = gi*256 + c, 1024 per half)
            h_t = hp.tile([P, M // P, 4 * CAP], bf16, tag="h_t")
            for mi in range(M // P):
                wib_t = wip.tile([P, D // P, P], bf16, tag="wib_t")
                nc.sync.dma_start(
                    wib_t[:],
                    wib[:, mi * P : (mi + 1) * P].rearrange("(dc p) m -> p dc m", p=P),
                )
                for pr in range(2):
                    psh = psum.tile([P, 2 * CAP], f32, tag="bank")
                    for dc in range(D // P):
                        nc.tensor.matmul(
                            psh[:], wib_t[:, dc, :], xeT[:, dc, pr * 2 : pr * 2 + 2, :],
                            start=(dc == 0), stop=(dc == D // P - 1),
                        )
                    nc.scalar.activation(
                        h_t[:, mi, pr * 512 : (pr + 1) * 512], psh[:],
                        Act.Gelu_apprx_tanh, bias=bi_sb[:, mi : mi + 1],
                    )

            # GEMM2 and combine, 2 sets x 512 tokens
            for st in range(2):
                psye = [
                    [
                        psum.tile([P, 512], f32, tag="bank", name=f"ye{hh}_{st}_{tc_}_{dt}")
                        for dt in range(2)
                    ]
                    for tc_ in range(4)
                ]
                for mi in range(M // P):
                    wob_t = wop.tile([P, D], bf16, tag="wob_t")
                    nc.sync.dma_start(wob_t[:], wob[mi * P : (mi + 1) * P, :])
                    for tc_ in range(4):
                        for dt in range(2):
                            nc.tensor.matmul(
                                psye[tc_][dt][:],
                                h_t[:, mi, st * 512 + tc_ * P : st * 512 + (tc_ + 1) * P],
                                wob_t[:, dt * 512 : (dt + 1) * 512],
                                start=(mi == 0), stop=(mi == M // P - 1),
                            )
                ye_sb = yep.tile([P, 4, D], bf16, tag="ye_sb")
                for tc_ in range(4):
                    for dt in range(2):
                        nc.any.tensor_copy(
                            ye_sb[:, tc_, dt * 512 : (dt + 1) * 512], psye[tc_][dt][:]
                        )
                for g2 in range(2):
                    g = 4 * hh + 2 * st + g2
                    cT = ctp.tile([P, 2, GS], bf16, tag="cT")
                    for j in range(8):
                        cb = cbp.tile([P, CAP], bf16, tag="cb")
                        nc.vector.tensor_scalar(
                            cb[:], iota256[:], poscols[:, j, g : g + 1], None,
                            op0=Alu.is_equal,
                        )
                        nc.vector.tensor_scalar_mul(cb[:], cb[:], wcols[:, j, g : g + 1])
                        for cc in range(2):
                            pcb = psum.tile([P, P], bf16, tag="bank")
                            nc.tensor.transpose(
                                pcb[:], cb[:, cc * P : (cc + 1) * P], I128b[:]
                            )
                            nc.any.tensor_copy(cT[:, cc, j * P : (j + 1) * P], pcb[:])
                    for j in range(8):
                        yt = yop.tile([P, D], f32, tag="yt")
                        for dt in range(2):
                            psy = psum.tile([P, 512], f32, tag="bank")
                            for cc in range(2):
                                nc.tensor.matmul(
                                    psy[:],
                                    cT[:, cc, j * P : (j + 1) * P],
                                    ye_sb[:, g2 * 2 + cc, dt * 512 : (dt + 1) * 512],
                                    start=(cc == 0), stop=(cc == 1),
                                )
                            nc.vector.scalar_tensor_tensor(
                                yt[:, dt * 512 : (dt + 1) * 512],
                                in0=bo_b[:, dt * 512 : (dt + 1) * 512],
                                scalar=wcols[:, j, g : g + 1], in1=psy[:],
                                op0=Alu.mult, op1=Alu.add,
                            )
                        nc.sync.dma_start(
                            y[g * GS + j * P : g * GS + (j + 1) * P, :], yt[:]
                        )

        for pool in (yop, cbp, ctp, yep, wop, wip, dpp, xbp, hp, xep, psum, pers, cst):
            pool.release()

    nc.compile()
    return nc


def _get_nc():
    if "nc" not in _CACHE:
        _CACHE["nc"] = _build_nc()
    return _CACHE["nc"]


def kernel(**inputs):
    from concourse.bass_utils import run_bass_kernel_spmd

    nc = _get_nc()

    x = np.ascontiguousarray(np.asarray(inputs["inputs"], dtype=np.float32)).reshape(NT, D)
    w_router = np.ascontiguousarray(np.asarray(inputs["w_router"], dtype=np.float32))
    wi = np.asarray(inputs["wi"], dtype=np.float32)
    bi = np.asarray(inputs["bi"], dtype=np.float32)
    wo = np.asarray(inputs["wo"], dtype=np.float32)
    bo = np.asarray(inputs["bo"], dtype=np.float32)

    x_hi = x.astype(ml_dtypes.bfloat16)
    x_t = np.ascontiguousarray(x.T)
    wi_bf = wi.astype(ml_dtypes.bfloat16)
    wo_bf = wo.astype(ml_dtypes.bfloat16)

    in_maps = []
    for e in range(N_CORES):
        in_maps.append(
            {
                "xhi": x_hi,
                "xt32": x_t,
                "wr": w_router,
                "wib": np.ascontiguousarray(wi_bf[e]),
                "bi": np.ascontiguousarray(bi[e]).reshape(1, M),
                "wob": np.ascontiguousarray(wo_bf[e]),
                "bo": np.ascontiguousarray(bo[e]).reshape(1, D),
                "eid": np.full((1, 1), float(e), dtype=np.float32),
            }
        )

    trace = bool(_CACHE.get("trace", False))
    res = None
    for attempt in range(3):
        try:
            res = run_bass_kernel_spmd(
                nc, in_maps, core_ids=list(range(N_CORES)), trace=trace,
                tmpdir=_CACHE.get("tmpdir"),
            )
            break
        except Exception:
            if attempt == 2:
                raise
    _CACHE["exec_time_ns"] = res.exec_time_ns

    y = res.results[0]["y"].astype(np.float64)
    for e in range(1, N_CORES):
        y += res.results[e]["y"]
    y = y.astype(np.float32).reshape(np.asarray(inputs["inputs"]).shape)
    aux = np.float32(res.results[0]["aux"][0, 0])
    return y, aux


# revision 12
# speedup vs baseline: 1.4453x; 1.0024x over previous
"""MoE (top-2, GShard-style capacity routing) kernel for 8 Trainium2 NeuronCores.

Expert-parallel: core e owns expert e's MLP (wi/wo/bi/bo slices). Every core
receives the full token stream, computes the (replicated, cheap) router +
top-2 + capacity assignment on device, dispatches its own expert's tokens via
a one-hot matmul, runs the expert MLP in bf16, and scatters back a weighted
partial output y_e. The host sums the 8 partials and takes the aux loss from
core 0.

Router precision: x and w_router are split on the host into bf16 hi/lo planes
(x = xh + xl with |x - xh - xl| <= 2^-18 |x|); logits are computed as
xh@wh + xh@wl + xl@wh in fp32 PSUM, giving ~2^-18-accurate logits so top-k
decisions match the fp32 reference. The bf16 hi planes are reused as the
dispatch/GEMM activations. xT tiles come in via DMA XBAR transpose (2-byte),
which keeps the PE free of fp32 transposes.

Self-contained: hardcodes all shapes; only imports system-installed
concourse/bass.
"""

import numpy as np
import ml_dtypes

# Problem shapes
E = 8
KSEL = 2
D = 1024
M = 4096
GS = 1024
G = 8
CAP = 256
NT = G * GS  # 8192 tokens
N_CORES = 8
P = 128

_CACHE = {}


def _build_nc():
    import concourse.bass as bass
    import concourse.mybir as mybir
    from concourse import bacc
    import concourse.tile as tile
    from concourse.masks import make_identity

    f32 = mybir.dt.float32
    bf16 = mybir.dt.bfloat16
    u32 = mybir.dt.uint32
    Alu = mybir.AluOpType
    Act = mybir.ActivationFunctionType
    Ax = mybir.AxisListType

    nc = bacc.Bacc(None, target_bir_lowering=False, debug=False)

    xhi = nc.declare_dram_parameter("xhi", [NT, D], bf16, isOutput=False)
    xt16h = nc.declare_dram_parameter("xt16h", [D, NT], mybir.dt.float16, isOutput=False)
    xt16l = nc.declare_dram_parameter("xt16l", [D, NT], mybir.dt.float16, isOutput=False)
    wrh = nc.declare_dram_parameter("wrh", [D, E], mybir.dt.float16, isOutput=False)
    wrl = nc.declare_dram_parameter("wrl", [D, E], mybir.dt.float16, isOutput=False)
    wib = nc.declare_dram_parameter("wib", [D, M], bf16, isOutput=False)
    bi_ = nc.declare_dram_parameter("bi", [1, M], f32, isOutput=False)
    wob = nc.declare_dram_parameter("wob", [M, D], bf16, isOutput=False)
    bo_ = nc.declare_dram_parameter("bo", [1, D], f32, isOutput=False)
    eid = nc.declare_dram_parameter("eid", [1, 1], f32, isOutput=False)
    y = nc.declare_dram_parameter("y", [NT, D], f32, isOutput=True)
    aux = nc.declare_dram_parameter("aux", [1, 1], f32, isOutput=True)

    iota_d = nc.inline_tensor(
        np.arange(CAP, dtype=np.float32).reshape(1, CAP), name="iotac"
    )

    with tile.TileContext(nc) as tc:
        cst = tc.alloc_tile_pool(name="cst", bufs=1)
        pers = tc.alloc_tile_pool(name="pers", bufs=1)
        psum = tc.alloc_tile_pool(name="psum", bufs=8, space="PSUM")

        I128f = cst.tile([P, P], f32, tag="I128f")
        make_identity(nc, I128f)
        I128b = cst.tile([P, P], bf16, tag="I128b")
        make_identity(nc, I128b)
        iota256 = cst.tile([P, CAP], f32, tag="iota256")
        nc.sync.dma_start(iota256[:], iota_d[:, :].to_broadcast((P, CAP)))
        bo_b = cst.tile([P, D], f32, tag="bo_b")
        nc.sync.dma_start(bo_b[:], bo_[:, :].to_broadcast((P, D)))
        ones_col = cst.tile([P, 1], f32, tag="ones_col")
        nc.vector.memset(ones_col[:], 1.0)
        bi_sb = cst.tile([P, M // P], f32, tag="bi_sb")
        nc.sync.dma_start(bi_sb[:], bi_[:, :].rearrange("a (mi p) -> p (a mi)", p=P))
        f16 = mybir.dt.float16
        wrh_sb = cst.tile([P, D // P, E], f16, tag="wrh_sb")
        nc.sync.dma_start(wrh_sb[:], wrh[:, :].rearrange("(dc p) e -> p dc e", p=P))
        wrl_sb = cst.tile([P, D // P, E], f16, tag="wrl_sb")
        nc.sync.dma_start(wrl_sb[:], wrl[:, :].rearrange("(dc p) e -> p dc e", p=P))
        ecol8 = cst.tile([G, 1], f32, tag="ecol8")
        nc.sync.dma_start(ecol8[:], eid[:, :].to_broadcast((G, 1)))

        poscols = pers.tile([P, 8, G], f32, tag="poscols")
        wcols = pers.tile([P, 8, G], f32, tag="wcols")
        impT = pers.tile([E, G], f32, tag="impT")

        # ------------------------------------------------------------------
        # Phase 1: router (split-bf16, fp32-accurate), top-2, capacity scan
        # ------------------------------------------------------------------
        rt = tc.alloc_tile_pool(name="rt", bufs=1)
        ph1 = tc.alloc_tile_pool(name="ph1", bufs=3)
        xtp = tc.alloc_tile_pool(name="xtp", bufs=2)

        rstage = rt.tile([4, NT], f32, tag="rstage")
        gatp = tc.alloc_tile_pool(name="gatp", bufs=12)
        imp_ps = {}
        ltTs = {}
        deferred = {}

        def softmax_block(st):
            ltT = ltTs.pop(st)
            defer = []
            for q in range(4):
                sc = st * 4 + q
                psl = psum.tile([P, E], f32, tag="bank")
                nc.tensor.transpose(psl[:], ltT[:, q * P : (q + 1) * P], I128f[:E, :E])
                lsb = ph1.tile([P, E], f32, tag="lsb")
                nc.vector.tensor_copy(lsb[:], psl[:])
                nm = ph1.tile([P, 1], f32, tag="nm")
                nc.vector.tensor_reduce(nm[:], lsb[:], axis=Ax.X, op=Alu.max, negate=True)
                m8 = ph1.tile([P, E], f32, tag="m8")
                nc.vector.max(m8[:], lsb[:])
                i8 = ph1.tile([P, E], u32, tag="i8")
                nc.vector.max_index(i8[:], m8[:], lsb[:])
                gat = gatp.tile([P, E], f32, tag="gat")
                se = ph1.tile([P, 1], f32, tag="se")
                nc.scalar.activation(gat[:], psl[:], Act.Exp, bias=nm[:], accum_out=se[:])
                rc = gatp.tile([P, 1], f32, tag="rc")
                nc.vector.reciprocal(rc[:], se[:])
                tw2 = ph1.tile([P, KSEL], f32, tag="tw2")
                nc.scalar.activation(tw2[:], m8[:, 0:KSEL], Act.Exp, bias=nm[:])
                pk = gatp.tile([P, 4], f32, tag="pk")
                nc.vector.tensor_copy(pk[:, 0:2], i8[:, 0:2])
                nc.vector.tensor_scalar_mul(pk[:, 2:4], tw2[:], rc[:])
                defer.append((sc, gat, rc, pk))
            deferred[st] = defer

        def pe_block(st):
            for sc, gat, rc, pk in deferred.pop(st):
                g, j = sc // 8, sc % 8
                if j == 0:
                    imp_ps[g] = psum.tile([E, 1], f32, tag="bank", name=f"imp{g}")
                # imp accumulates sum_s exp/sumexp via rhs = 1/sumexp column
                nc.tensor.matmul(
                    imp_ps[g][:], gat[:], rc[:], start=(j == 0), stop=(j == 7)
                )
                pspk = psum.tile([4, P], f32, tag="bank")
                nc.tensor.transpose(pspk[:], pk[:], I128f[:])
                nc.vector.tensor_copy(rstage[0:4, sc * P : (sc + 1) * P], pspk[:])
                if j == 7:
                    nc.vector.tensor_copy(impT[:, g : g + 1], imp_ps[g][:])

        for st in range(16):  # 512-token tiles, software-pipelined
            xTh = xtp.tile([P, D // P, 512], f16, tag="xTh")
            nc.scalar.dma_start(
                xTh[:],
                xt16h[:, st * 512 : (st + 1) * 512].rearrange("(dc p) s -> p dc s", p=P),
            )
            xTl = xtp.tile([P, D // P, 512], f16, tag="xTl")
            nc.scalar.dma_start(
                xTl[:],
                xt16l[:, st * 512 : (st + 1) * 512].rearrange("(dc p) s -> p dc s", p=P),
            )
            pslt = psum.tile([E, 512], f32, tag="bank")
            terms = ((wrh_sb, xTh), (wrl_sb, xTh), (wrh_sb, xTl))
            k = 0
            for dc in range(D // P):
                for wt, xt in terms:
                    nc.tensor.matmul(
                        pslt[:], wt[:, dc, :], xt[:, dc, :],
                        start=(k == 0), stop=(k == 3 * D // P - 1),
                    )
                    k += 1
            ltT = ph1.tile([E, 512], f32, tag="ltT")
            nc.vector.tensor_copy(ltT[:], pslt[:])
            ltTs[st] = ltT
            if st >= 1:
                softmax_block(st - 1)
            if st >= 2:
                pe_block(st - 2)
        softmax_block(15)
        pe_block(14)
        pe_block(15)
        gatp.release()

        # k-major stream [g, t=k*GS+s] and capacity scan
        topiT = rt.tile([G, KSEL * GS], f32, tag="topiT")
        twT = rt.tile([G, KSEL * GS], f32, tag="twT")
        nc.sync.dma_start(topiT[:, 0:GS], rstage[0:1, :])
        nc.sync.dma_start(topiT[:, GS : 2 * GS], rstage[1:2, :])
        nc.sync.dma_start(twT[:, 0:GS], rstage[2:3, :])
        nc.sync.dma_start(twT[:, GS : 2 * GS], rstage[3:4, :])
        zz8 = rt.tile([G, KSEL * GS], f32, tag="zz8")
        nc.vector.memset(zz8[:], 0.0)
        ohh = rt.tile([G, KSEL * GS], f32, tag="ohh")
        nc.vector.tensor_scalar(ohh[:], topiT[:], ecol8[:, 0:1], None, op0=Alu.is_equal)
        incl = rt.tile([G, KSEL * GS], f32, tag="incl")
        nc.vector.tensor_tensor_scan(incl[:], ohh[:], zz8[:], 0.0, op0=Alu.add, op1=Alu.add)
        pos = rt.tile([G, KSEL * GS], f32, tag="pos")
        nc.vector.tensor_tensor(pos[:], incl[:], ohh[:], Alu.subtract)
        keep = rt.tile([G, KSEL * GS], f32, tag="keep")
        nc.vector.scalar_tensor_tensor(
            keep[:], in0=pos[:], scalar=float(CAP), in1=ohh[:],
            op0=Alu.is_lt, op1=Alu.mult,
        )
        posm = rt.tile([G, KSEL * GS], f32, tag="posm")
        nc.vector.scalar_tensor_tensor(
            posm[:], in0=pos[:], scalar=999.0, in1=keep[:],
            op0=Alu.subtract, op1=Alu.mult,
        )
        nc.vector.tensor_scalar_add(posm[:], posm[:], 999.0)
        wsel = rt.tile([G, KSEL * GS], f32, tag="wsel")
        nc.vector.tensor_tensor(wsel[:], twT[:], keep[:], Alu.mult)
        posm_tok = rt.tile([G, GS], f32, tag="posm_tok")
        nc.vector.tensor_tensor(posm_tok[:], posm[:, 0:GS], posm[:, GS : 2 * GS], Alu.min)
        wtok = rt.tile([G, GS], f32, tag="wtok")
        nc.vector.tensor_tensor(wtok[:], wsel[:, 0:GS], wsel[:, GS : 2 * GS], Alu.add)
        for j in range(8):
            p1 = psum.tile([P, G], f32, tag="bank")
            nc.tensor.transpose(p1[:], posm_tok[:, j * P : (j + 1) * P], I128f[:G, :G])
            nc.vector.tensor_copy(poscols[:, j, :], p1[:])
            p2 = psum.tile([P, G], f32, tag="bank")
            nc.tensor.transpose(p2[:], wtok[:, j * P : (j + 1) * P], I128f[:G, :G])
            nc.vector.tensor_copy(wcols[:, j, :], p2[:])

        # aux = mean_g (std_e(imp)/mean_e(imp))^2
        pst = psum.tile([G, E], f32, tag="bank")
        nc.tensor.transpose(pst[:], impT[:], I128f[:E, :E])
        imp_ge = pers.tile([G, E], f32, tag="imp_ge")
        nc.vector.tensor_copy(imp_ge[:], pst[:])
        mu = pers.tile([G, 1], f32, tag="mu")
        nc.vector.tensor_reduce(mu[:], imp_ge[:], axis=Ax.X, op=Alu.add)
        nc.vector.tensor_scalar_mul(mu[:], mu[:], 1.0 / E)
        dif = pers.tile([G, E], f32, tag="dif")
        nc.vector.tensor_scalar(dif[:], imp_ge[:], mu[:, 0:1], None, op0=Alu.subtract)
        nc.vector.tensor_tensor(dif[:], dif[:], dif[:], Alu.mult)
        var = pers.tile([G, 1], f32, tag="var")
        nc.vector.tensor_reduce(var[:], dif[:], axis=Ax.X, op=Alu.add)
        nc.vector.tensor_scalar_mul(var[:], var[:], 1.0 / E)
        mu2 = pers.tile([G, 1], f32, tag="mu2")
        nc.vector.tensor_tensor(mu2[:], mu[:], mu[:], Alu.mult)
        rr = pers.tile([G, 1], f32, tag="rr")
        nc.vector.reciprocal(rr[:], mu2[:])
        ratio = pers.tile([G, 1], f32, tag="ratio")
        nc.vector.tensor_tensor(ratio[:], var[:], rr[:], Alu.mult)
        psa = psum.tile([1, G], f32, tag="bank")
        nc.tensor.transpose(psa[:], ratio[:], I128f[:G, :G])
        arow = pers.tile([1, G], f32, tag="arow")
        nc.vector.tensor_copy(arow[:], psa[:])
        auxv = pers.tile([1, 1], f32, tag="auxv")
        nc.vector.tensor_reduce(auxv[:], arow[:], axis=Ax.X, op=Alu.add)
        nc.vector.tensor_scalar_mul(auxv[:], auxv[:], 1.0 / G)
        nc.sync.dma_start(aux[:, :], auxv[:])

        xtp.release()
        ph1.release()
        rt.release()

        # ------------------------------------------------------------------
        # Phase 2: dispatch -> MLP -> combine, two halves of 4 groups
        # ------------------------------------------------------------------
        xep = tc.alloc_tile_pool(name="xep", bufs=1)
        hp = tc.alloc_tile_pool(name="hp", bufs=1)
        xbp = tc.alloc_tile_pool(name="xbp", bufs=3)
        dpp = tc.alloc_tile_pool(name="dpp", bufs=3)
        wip = tc.alloc_tile_pool(name="wip", bufs=3)
        wop = tc.alloc_tile_pool(name="wop", bufs=3)
        yep = tc.alloc_tile_pool(name="yep", bufs=1)
        ctp = tc.alloc_tile_pool(name="ctp", bufs=2)
        cbp = tc.alloc_tile_pool(name="cbp", bufs=3)
        yop = tc.alloc_tile_pool(name="yop", bufs=3)

        for hh in range(2):
            # dispatch: xeT[d, dc, gi, c] = x^T gathered per capacity slot
            xeT = xep.tile([P, D // P, 4, CAP], bf16, tag="xeT")
            for gi in range(4):
                g = 4 * hh + gi
                psxe = [
                    psum.tile([P, CAP], f32, tag="bank", name=f"xe{g}_{dc}")
                    for dc in range(D // P)
                ]
                for jj in range(4):
                    xb = xbp.tile([P, 2, D], bf16, tag="xb")
                    base = (g * 8 + jj * 2) * P
                    nc.sync.dma_start(
                        xb[:],
                        xhi[base : base + 2 * P, :].rearrange("(two p) d -> p two d", p=P),
                    )
                    for j2 in range(2):
                        j = jj * 2 + j2
                        dp = dpp.tile([P, CAP], bf16, tag="dp")
                        nc.vector.tensor_scalar(
                            dp[:], iota256[:], poscols[:, j, g : g + 1], None,
                            op0=Alu.is_equal,
                        )
                        for dc in range(D // P):
                            nc.tensor.matmul(
                                psxe[dc][:], xb[:, j2, dc * P : (dc + 1) * P], dp[:],
                                start=(j == 0), stop=(j == 7),
                            )
                for dc in range(D // P):
                    nc.any.tensor_copy(xeT[:, dc, gi, :], psxe[dc][:])

            # GEMM1 + bias + gelu -> h[m, mi, t]  (t = gi*256 + c, 1024 per half)
            h_t = hp.tile([P, M // P, 4 * CAP], bf16, tag="h_t")
            for mi in range(M // P):
                wib_t = wip.tile([P, D // P, P], bf16, tag="wib_t")
                nc.sync.dma_start(
                    wib_t[:],
                    wib[:, mi * P : (mi + 1) * P].rearrange("(dc p) m -> p dc m", p=P),
                )
                for pr in range(2):
                    psh = psum.tile([P, 2 * CAP], f32, tag="bank")
                    for dc in range(D // P):
                        nc.tensor.matmul(
                            psh[:], wib_t[:, dc, :], xeT[:, dc, pr * 2 : pr * 2 + 2, :],
                            start=(dc == 0), stop=(dc == D // P - 1),
                        )
                    nc.scalar.activation(
                        h_t[:, mi, pr * 512 : (pr + 1) * 512], psh[:],
                        Act.Gelu_apprx_tanh, bias=bi_sb[:, mi : mi + 1],
                    )

            # GEMM2 and combine, 2 sets x 512 tokens
            for st in range(2):
                psye = [
                    [
                        psum.tile([P, 512], f32, tag="bank", name=f"ye{hh}_{st}_{tc_}_{dt}")
                        for dt in range(2)
                    ]
                    for tc_ in range(4)
                ]
                for mi in range(M // P):
                    wob_t = wop.tile([P, D], bf16, tag="wob_t")
                    nc.sync.dma_start(wob_t[:], wob[mi * P : (mi + 1) * P, :])
                    for tc_ in range(4):
                        for dt in range(2):
                            nc.tensor.matmul(
                                psye[tc_][dt][:],
                                h_t[:, mi, st * 512 + tc_ * P : st * 512 + (tc_ + 1) * P],
                                wob_t[:, dt * 512 : (dt + 1) * 512],
                                start=(mi == 0), stop=(mi == M // P - 1),
                            )
                ye_sb = yep.tile([P, 4, D], bf16, tag="ye_sb")
                for tc_ in range(4):
                    for dt in range(2):
                        nc.any.tensor_copy(
                            ye_sb[:, tc_, dt * 512 : (dt + 1) * 512], psye[tc_][dt][:]
                        )
                for g2 in range(2):
                    g = 4 * hh + 2 * st + g2
                    cT = ctp.tile([P, 2, GS], bf16, tag="cT")
                    for j in range(8):
                        cb = cbp.tile([P, CAP], bf16, tag="cb")
                        nc.vector.tensor_scalar(
                            cb[:], iota256[:], poscols[:, j, g : g + 1], None,
                            op0=Alu.is_equal,
                        )
                        nc.vector.tensor_scalar_mul(cb[:], cb[:], wcols[:, j, g : g + 1])
                        for cc in range(2):
                            pcb = psum.tile([P, P], bf16, tag="bank")
                            nc.tensor.transpose(
                                pcb[:], cb[:, cc * P : (cc + 1) * P], I128b[:]
                            )
                            nc.any.tensor_copy(cT[:, cc, j * P : (j + 1) * P], pcb[:])
                    for j in range(8):
                        yt = yop.tile([P, D], f32, tag="yt")
                        for dt in range(2):
                            psy = psum.tile([P, 512], f32, tag="bank")
                            for cc in range(2):
                                nc.tensor.matmul(
                                    psy[:],
                                    cT[:, cc, j * P : (j + 1) * P],
                                    ye_sb[:, g2 * 2 + cc, dt * 512 : (dt + 1) * 512],
                                    start=(cc == 0), stop=(cc == 1),
                                )
                            nc.vector.scalar_tensor_tensor(
                                yt[:, dt * 512 : (dt + 1) * 512],
                                in0=bo_b[:, dt * 512 : (dt + 1) * 512],
                                scalar=wcols[:, j, g : g + 1], in1=psy[:],
                                op0=Alu.mult, op1=Alu.add,
                            )
                        nc.sync.dma_start(
                            y[g * GS + j * P : g * GS + (j + 1) * P, :], yt[:]
                        )

        for pool in (yop, cbp, ctp, yep, wop, wip, dpp, xbp, hp, xep, psum, pers, cst):
            pool.release()

    nc.compile()
    return nc


def _get_nc():
    if "nc" not in _CACHE:
        _CACHE["nc"] = _build_nc()
    return _CACHE["nc"]


def kernel(**inputs):
    from concourse.bass_utils import run_bass_kernel_spmd

    nc = _get_nc()

    x = np.ascontiguousarray(np.asarray(inputs["inputs"], dtype=np.float32)).reshape(NT, D)
    w_router = np.ascontiguousarray(np.asarray(inputs["w_router"], dtype=np.float32))
    wi = np.asarray(inputs["wi"], dtype=np.float32)
    bi = np.asarray(inputs["bi"], dtype=np.float32)
    wo = np.asarray(inputs["wo"], dtype=np.float32)
    bo = np.asarray(inputs["bo"], dtype=np.float32)

    x_hi = x.astype(ml_dtypes.bfloat16)
    x_t = np.ascontiguousarray(x.T)
    xt_h = x_t.astype(np.float16)
    xt_l = (x_t - xt_h.astype(np.float32)).astype(np.float16)
    wr_h = w_router.astype(np.float16)
    wr_l = (w_router - wr_h.astype(np.float32)).astype(np.float16)
    wi_bf = wi.astype(ml_dtypes.bfloat16)
    wo_bf = wo.astype(ml_dtypes.bfloat16)

    in_maps = []
    for e in range(N_CORES):
        in_maps.append(
            {
                "xhi": x_hi,
                "xt16h": xt_h,
                "xt16l": xt_l,
                "wrh": wr_h,
                "wrl": wr_l,
                "wib": np.ascontiguousarray(wi_bf[e]),
                "bi": np.ascontiguousarray(bi[e]).reshape(1, M),
                "wob": np.ascontiguousarray(wo_bf[e]),
                "bo": np.ascontiguousarray(bo[e]).reshape(1, D),
                "eid": np.full((1, 1), float(e), dtype=np.float32),
            }
        )

    trace = bool(_CACHE.get("trace", False))
    res = None
    for attempt in range(3):
        try:
            res = run_bass_kernel_spmd(
                nc, in_maps, core_ids=list(range(N_CORES)), trace=trace,
                tmpdir=_CACHE.get("tmpdir"),
            )
            break
        except Exception:
            if attempt == 2:
                raise
    _CACHE["exec_time_ns"] = res.exec_time_ns

    y = res.results[0]["y"].astype(np.float64)
    for e in range(1, N_CORES):
        y += res.results[e]["y"]
    y = y.astype(np.float32).reshape(np.asarray(inputs["inputs"]).shape)
    aux = np.float32(res.results[0]["aux"][0, 0])
    return y, aux
